# revision 13
# baseline (speedup 1.0000x reference)
"""Causal self-attention (B=2, S=2048, E=2048, H=16) on 8 TRN2 NeuronCores.

Sharding: core = 4*b + hg  (b in {0,1} data-parallel over batch,
hg in {0..3} tensor-parallel over groups of 4 heads / Wqkv columns /
Wo rows).  Each core computes a partial [S, E] output (its heads'
contribution through Wo); the host sums the 4 partials per batch.

v2 design (all-bf16, SBUF-resident, software-pipelined):
  - Every matmul operand is bf16 (PSUM accumulation stays fp32), which
    halves DMA traffic + SBUF footprint vs f32r and enables FWL (2x
    faster LDWEIGHTS), keeping the weight path off the PE critical path.
  - No DRAM spill: x, q/k (post-RoPE), v, oT all live in SBUF; q/k
    weights are host-pre-transposed so every DMA is contiguous.
  - Softmax denominator is NOT computed with per-block ones-matmuls
    (which would double the AV cost on PE).  Exp blocks are summed by a
    sequential bf16 chain on the DVE (each 438ns link is faster than
    the ~660ns exp cadence, so the chain never lags), then one small
    bf16 ones-matmul per (head, window) broadcasts the partition sum.
  - RoPE: rot = raw*cos + pairswap(raw*sin_signed), with the rotate-
    half signs folded into sin on the host so the swap is a pure
    partition-pair permutation done by two strided SBUF->SBUF DMAs —
    zero PE cost.
  - Software pipelining: attention of head h (ACT-heavy exp, PE-light)
    is emitted as a thunk stream interleaved into the projection matmul
    stream of head h+1 and the final Wo stream, so the PE never drains
    while the scalar engine chews through exp.  Measured: single >0.5us
    PE gap over the whole kernel outside the DMA preamble/tail.
"""
import sys

if "/opt/trn_rl_repo" not in sys.path:
    sys.path.insert(0, "/opt/trn_rl_repo")

from collections import deque
from contextlib import ExitStack

import numpy as np
import ml_dtypes

import concourse.bass as bass
import concourse.mybir as mybir
import concourse.tile as tile
from concourse import bacc, bass_utils

F32 = mybir.dt.float32
F32R = mybir.dt.float32r
BF16 = mybir.dt.bfloat16
AF = mybir.ActivationFunctionType
ALU = mybir.AluOpType

B = 2
S = 2048
E = 2048
H = 16
D = 128
HL = 4            # heads per core
P = 128
NE = E // P       # 16 contraction chunks
NW = S // 512     # 4 q windows of 512
NSC = S // P      # 16 s-chunks
W512 = 512
NEG = -1.0e30

_PROGRAM = None


def _build_program():
    nc = bacc.Bacc("TRN2", target_bir_lowering=False, debug=False, num_devices=8)

    # x^T split column-wise into contiguous host tensors so the v-phase
    # e-loop is PE-bound, not DMA-bound (first pass needs only [0:512])
    xq1 = nc.dram_tensor("xq1", [E, W512], BF16, kind="ExternalInput").ap()
    xq2 = nc.dram_tensor("xq2", [E, W512], BF16, kind="ExternalInput").ap()
    xh2 = nc.dram_tensor("xh2", [E, 2 * W512], BF16, kind="ExternalInput").ap()
    # pre-transposed on host: wq[h, p, e*D+c] = Wq_scaled[e*P+p, h*D+c]
    wq = nc.dram_tensor("wq", [HL, P, NE * D], BF16, kind="ExternalInput").ap()
    wk = nc.dram_tensor("wk", [HL, P, NE * D], BF16, kind="ExternalInput").ap()
    wv = nc.dram_tensor("wv", [E, HL * D], BF16, kind="ExternalInput").ap()
    wo = nc.dram_tensor("wo", [HL * D, E], BF16, kind="ExternalInput").ap()
    cosf = nc.dram_tensor("cosf", [P, S], BF16, kind="ExternalInput").ap()
    sinf = nc.dram_tensor("sinf", [P, S], BF16, kind="ExternalInput").ap()
    maskT = nc.dram_tensor("maskT", [P, 4 * W512], BF16, kind="ExternalInput").ap()
    outp = nc.dram_tensor("outp", [S, E], BF16, kind="ExternalOutput").ap()

    with tile.TileContext(nc) as tc:
        with ExitStack() as ctx_all:
            cpool = ctx_all.enter_context(tc.tile_pool(name="const", bufs=1))
            ones_t = cpool.tile([P, P], BF16, tag="ones")
            # ones synthesized on-chip (no DMA); warm tile feeds the PE
            # warm-up matmuls that ramp the clock during the ~8.6us DMA
            # ring-init dead zone before any input data can arrive.
            warm_t = cpool.tile([P, W512], BF16, tag="warm")
            nc.gpsimd.memset(ones_t[:], 1.0)
            nc.gpsimd.memset(warm_t[:], 1.0)

            vsb_pool = ctx_all.enter_context(tc.tile_pool(name="vsb", bufs=1))
            vsb = vsb_pool.tile([P, NSC, HL, D], BF16, tag="v")
            qk_pool = ctx_all.enter_context(tc.tile_pool(name="qk", bufs=1))
            qT = [qk_pool.tile([P, S], BF16, tag=f"q{h}", name=f"qT_{h}") for h in range(HL)]
            kT = [qk_pool.tile([P, S], BF16, tag=f"k{h}", name=f"kT_{h}") for h in range(HL)]
            oT_pool = ctx_all.enter_context(tc.tile_pool(name="oT", bufs=1))
            oTt = [oT_pool.tile([P, S], BF16, tag=f"o{h}", name=f"oT_{h}") for h in range(HL)]

            # ------- attention-stream machinery (emitted via pump()) -------
            pending = deque()

            def pump(n=1):
                for _ in range(n):
                    if not pending:
                        return
                    pending.popleft()()

            def drain():
                while pending:
                    pending.popleft()()

            # the following pools are entered later (after the v-phase frees
            # its SBUF); placeholders filled by alloc_attn_pools()
            attn = {}

            def alloc_attn_pools():
                attn["mask_pool"] = ctx_all.enter_context(
                    tc.tile_pool(name="mask", bufs=1, side="right"))
                attn["mask_t"] = attn["mask_pool"].tile([P, 4 * W512], BF16,
                                                        tag="mask", name="mask_t")
                nc.sync.dma_start(attn["mask_t"][:], maskT)
                attn["expw_pool"] = ctx_all.enter_context(
                    tc.tile_pool(name="expw", bufs=1, side="right"))
                attn["acc_pool"] = ctx_all.enter_context(
                    tc.tile_pool(name="acc", bufs=1, side="right"))
                attn["rec_pool"] = ctx_all.enter_context(
                    tc.tile_pool(name="rec", bufs=1, side="right"))
                attn["ps_s"] = ctx_all.enter_context(
                    tc.tile_pool(name="ps_s", bufs=4, space="PSUM", side="right"))
                attn["ps_o"] = ctx_all.enter_context(
                    tc.tile_pool(name="ps_o", bufs=2, space="PSUM", side="right"))

            # per-head transient state for the in-flight stream
            st = {}

            def op_score(h, w, c):
                def f():
                    r = c - 4 * w
                    ex0 = max(0, P * r)
                    pss = attn["ps_s"].tile([P, W512], F32, tag="s", name=f"pss_{h}_{w}_{c}")
                    nc.tensor.matmul(
                        pss[:, ex0:], kT[h][:, c * P:(c + 1) * P],
                        qT[h][:, w * W512 + ex0:(w + 1) * W512],
                        start=True, stop=True)
                    if r >= 0:
                        m = attn["mask_t"]
                        nc.vector.tensor_add(
                            pss[:, ex0:ex0 + P], pss[:, ex0:ex0 + P],
                            m[:, r * W512 + ex0:r * W512 + ex0 + P])
                    expw = st[(h, "expw", w)]
                    if ex0 > 0:
                        nc.gpsimd.memset(expw[:, c, 0:ex0], 0.0)
                    nc.scalar.activation(expw[:, c, ex0:], pss[:, ex0:], AF.Exp)
                    # denominator partial sum: sequential bf16 chain on DVE
                    # (438ns/link beats the ~660ns exp cadence, so the chain
                    # never lags the exp stream)
                    if c == 0:
                        acc = attn["acc_pool"].tile([P, W512], BF16, tag="a",
                                                    name=f"acc_{h}_{w}")
                        st[(h, "acc", w)] = acc
                        nc.vector.tensor_copy(acc[:], expw[:, 0, :])
                    else:
                        acc = st[(h, "acc", w)]
                        nc.vector.tensor_add(acc[:], acc[:], expw[:, c, :])
                return f

            def op_expw_alloc(h, w):
                def f():
                    nblk = 4 * w + 4
                    st[(h, "expw", w)] = attn["expw_pool"].tile(
                        [P, nblk, W512], BF16, tag=f"e{w}", name=f"expw_{h}_{w}")
                return f

            def op_av(h, w, c):
                def f():
                    nblk = 4 * w + 4
                    r = c - 4 * w
                    ex0 = max(0, P * r)
                    if c == 0:
                        st[(h, "pso", w)] = attn["ps_o"].tile(
                            [P, W512], F32, tag="o", name=f"pso_{h}_{w}")
                    pso = st[(h, "pso", w)]
                    expw = st[(h, "expw", w)]
                    nc.tensor.matmul(
                        pso[:, ex0:], vsb[:, c, h, :], expw[:, c, ex0:],
                        start=(c == 0), stop=(c == nblk - 1))
                return f

            def op_dm(h, w, done_marker=None):
                def f():
                    acc = st.pop((h, "acc", w))
                    pso = st.pop((h, "pso", w))
                    psd = attn["ps_s"].tile([P, W512], F32, tag="s", name=f"psd_{h}_{w}")
                    nc.tensor.matmul(psd[:], ones_t[:], acc[:],
                                     start=True, stop=True)
                    recd = attn["rec_pool"].tile([P, W512], F32, tag="r", name=f"recd_{h}_{w}")
                    nc.vector.reciprocal_approx_fast(out=recd[:], in_=psd[:])
                    nc.vector.tensor_mul(
                        oTt[h][:, w * W512:(w + 1) * W512], pso[:], recd[:])
                    if done_marker is not None:
                        done_marker[0] = True
                return f

            # dm_done[w]: Wo phase drains the h3 stream until these flip
            dm_done = {w: [False] for w in range(NW)}

            def stream_ops_for_window(h, w, mark):
                """ops that become emittable once proj window w of head h
                is complete (rot written).  Returns list of thunks."""
                ops = []
                ops.append(op_expw_alloc(h, w))
                for c in range(4 * w + 4):
                    ops.append(op_score(h, w, c))
                if w >= 1:
                    wp = w - 1
                    for c in range(4 * wp + 4):
                        ops.append(op_av(h, wp, c))
                    ops.append(op_dm(h, wp, mark[wp] if mark else None))
                if w == NW - 1:
                    for c in range(4 * w + 4):
                        ops.append(op_av(h, w, c))
                    ops.append(op_dm(h, w, mark[w] if mark else None))
                return ops

            def enqueue_head(h, mark=None):
                for w in range(NW):
                    pending.extend(stream_ops_for_window(h, w, mark))

            # ---------------- Phase 1: QKV projection + RoPE ----------------
            with ExitStack() as ctx1:
                xpool = ctx1.enter_context(tc.tile_pool(name="xT", bufs=NE))
                cspool = ctx1.enter_context(tc.tile_pool(name="cossin", bufs=1))
                wqk_pool = ctx1.enter_context(tc.tile_pool(name="wqk", bufs=4))
                rsin_pool = ctx1.enter_context(tc.tile_pool(name="rsin", bufs=4))

                xt = []
                with ExitStack() as ctxv:
                    wv_pool = ctxv.enter_context(tc.tile_pool(name="wv", bufs=NE))
                    ps_v = ctxv.enter_context(
                        tc.tile_pool(name="ps_v", bufs=8, space="PSUM"))
                    # triple-interleave (wv_e, xq1_e, xq2_e) so the 8-chunk
                    # v pass (1.73us of PE per triple) chases the DMA stream
                    # PE-bound instead of DMA-starved
                    wvt = []
                    for e in range(NE):
                        tw = wv_pool.tile([P, HL * D], BF16, tag="wv",
                                          name=f"wv_{e}")
                        nc.sync.dma_start(tw[:], wv[e * P:(e + 1) * P, :])
                        wvt.append(tw)
                        t1 = xpool.tile([P, W512], BF16, tag="x1", name=f"x1_{e}")
                        nc.sync.dma_start(t1[:], xq1[e * P:(e + 1) * P, :])
                        t2 = xpool.tile([P, W512], BF16, tag="x2", name=f"x2_{e}")
                        nc.sync.dma_start(t2[:], xq2[e * P:(e + 1) * P, :])
                        xt.append([t1, t2, None])
                    for e in range(NE):
                        t3 = xpool.tile([P, 2 * W512], BF16, tag="x3",
                                        name=f"x3_{e}")
                        nc.sync.dma_start(t3[:], xh2[e * P:(e + 1) * P, :])
                        xt[e][2] = t3
                    # q/k weights for head 0 prefetched during the v-phase
                    wt_qk = {}
                    for m, wsrc in ((0, wq), (1, wk)):
                        t = wqk_pool.tile([P, NE, P], BF16, tag="wqk", name=f"wt_0_{m}")
                        nc.sync.dma_start(
                            t[:].rearrange("p n c -> p (n c)"), wsrc[0])
                        wt_qk[(0, m)] = t
                    cos_t = cspool.tile([P, S], BF16, tag="cos")
                    nc.sync.dma_start(cos_t[:], cosf)
                    sin_t = cspool.tile([P, S], BF16, tag="sin")
                    nc.sync.dma_start(sin_t[:], sinf)

                    # PE warm-up: dummy matmuls with no DMA dependency ramp
                    # the clock through the DMA-init dead zone so the first
                    # real matmul runs at full p-state.
                    ps_warm = ps_v.tile([P, W512], F32, tag="v", name="ps_warm")
                    for _ in range(7):
                        nc.tensor.matmul(ps_warm[:], warm_t[:, 0:P], warm_t[:],
                                         start=True, stop=True)

                    # v: 2 passes of 8 s-chunks on all 8 PSUM banks.  The
                    # first/last accumulation rounds run ci-descending and the
                    # psum->vsb copies are spread over Scalar/Vector/GpSimd in
                    # completion order, so the banks that the NEXT consumer
                    # (pass-1 round 0 / first qk matmul) grabs are already
                    # free when it arrives.
                    for g in range(2):
                        psvs = [ps_v.tile([P, W512], F32, tag="v",
                                          name=f"psv_{g}_{ci}")
                                for ci in range(8)]
                        for e in range(NE):
                            cis = range(7, -1, -1) if e in (0, NE - 1) else range(8)
                            for ci in cis:
                                c = g * 8 + ci
                                if c < 4:
                                    xs = xt[e][0][:, c * P:(c + 1) * P]
                                elif c < 8:
                                    xs = xt[e][1][:, (c - 4) * P:(c - 3) * P]
                                else:
                                    xs = xt[e][2][:, (c - 8) * P:(c - 7) * P]
                                nc.tensor.matmul(
                                    psvs[ci][:], xs, wvt[e][:],
                                    start=(e == 0), stop=(e == NE - 1))
                        for ci in range(7, -1, -1):
                            c = g * 8 + ci
                            # [s, 4h*128] -> vsb[:, c, h, :]
                            dstv = vsb[:, c, :, :].rearrange("p h d -> p (h d)")
                            eng = (nc.scalar.copy,
                                   nc.vector.tensor_copy)[(7 - ci) % 2]
                            eng(dstv, psvs[ci][:])

                # v-phase SBUF/PSUM released.  attn pools are allocated one
                # window into head 0's projection (their gpsimd zero-fills and
                # the mask DMA then overlap qk matmuls instead of gating the
                # first one); only ps_qk/swp are needed immediately.
                ps_qk = ctx1.enter_context(
                    tc.tile_pool(name="ps_qk", bufs=2, space="PSUM"))
                swp_pool = ctx1.enter_context(tc.tile_pool(name="swp", bufs=2))

                # q, k projection with RoPE; per head: w-outer, m-inner.
                # rope "finish" (swap-matmul + combine) for window w is
                # deferred into window w+1's e-loop so the PE never waits
                # on the DVE.
                for h in range(HL):
                    rope_dma_q = deque()   # (rs, dst) awaiting the swap DMA
                    rope_add_q = deque()   # (swp, dst) awaiting the combine

                    def rope_dma():
                        rs_keep, dst = rope_dma_q.popleft()
                        swp = swp_pool.tile([P, W512], BF16, tag="sw",
                                            name="swp")
                        nc.sync.dma_start(swp[0::2, :], rs_keep[1::2, :])
                        nc.sync.dma_start(swp[1::2, :], rs_keep[0::2, :])
                        rope_add_q.append((swp, dst))

                    def rope_add():
                        swp, dst = rope_add_q.popleft()
                        nc.vector.tensor_add(dst, dst, swp[:])

                    def xwin(e, w):
                        if w == 0:
                            return xt[e][0][:]
                        if w == 1:
                            return xt[e][1][:]
                        return xt[e][2][:, (w - 2) * W512:(w - 1) * W512]

                    for w in range(NW):
                        ws = slice(w * W512, (w + 1) * W512)
                        for m in range(2):
                            wt = wt_qk[(h, m)]
                            ps = ps_qk.tile([P, W512], F32, tag="qk", name=f"psqk_{h}_{w}_{m}")
                            for e in range(NE):
                                nc.tensor.matmul(ps[:], wt[:, e, :],
                                                 xwin(e, w),
                                                 start=(e == 0),
                                                 stop=(e == NE - 1))
                                if e in (1, 2) and rope_dma_q:
                                    rope_dma()
                                elif e in (8, 9) and rope_add_q:
                                    rope_add()
                                elif e > 0:
                                    pump(2 if len(pending) > 50 else 1)
                            rs = rsin_pool.tile([P, W512], BF16, tag="rs", name=f"rs_{h}_{w}_{m}")
                            nc.vector.tensor_mul(rs[:], ps[:], sin_t[:, ws])
                            dst = (qT[h] if m == 0 else kT[h])[:, ws]
                            nc.vector.tensor_mul(dst, ps[:], cos_t[:, ws])
                            rope_dma_q.append((rs, dst))
                        if h == 0 and w == 0:
                            alloc_attn_pools()
                        # prefetch next head's weights well before its start
                        if h + 1 < HL and w == NW - 2:
                            for m2, wsrc in ((0, wq), (1, wk)):
                                t = wqk_pool.tile([P, NE, P], BF16, tag="wqk", name=f"wt_{h+1}_{m2}")
                                nc.sync.dma_start(
                                    t[:].rearrange("p n c -> p (n c)"),
                                    wsrc[h + 1])
                                wt_qk[(h + 1, m2)] = t
                        # head 3's attention ops become valid window by
                        # window (rope of window w finishes during window
                        # w+1): enqueue them as soon as they are legal so
                        # they drain during head 3's own projection.
                        if h == HL - 1 and w >= 1:
                            pending.extend(
                                stream_ops_for_window(h, w - 1, dm_done))
                    # flush the last window's rope
                    while rope_dma_q:
                        rope_dma()
                        pump(1)
                    while rope_add_q:
                        pump(2)
                        rope_add()
                    # head h attention becomes runnable; heads 0..2 are
                    # enqueued whole and pumped during head h+1's
                    # projection.  head 3's last windows drain in Wo.
                    if h == HL - 1:
                        pending.extend(
                            stream_ops_for_window(h, NW - 1, dm_done))
                    else:
                        enqueue_head(h)

            # ---------------- Phase 3: Wo ----------------
            with ExitStack() as ctx3:
                wo_pool = ctx3.enter_context(tc.tile_pool(name="wo", bufs=HL))
                ost_pool = ctx3.enter_context(tc.tile_pool(name="ost", bufs=2))
                ps_out = ctx3.enter_context(
                    tc.tile_pool(name="ps_out", bufs=2, space="PSUM"))
                wot = []
                for h2 in range(HL):
                    t = wo_pool.tile([P, E], BF16, tag="wo", name=f"wo_{h2}")
                    nc.sync.dma_start(t[:, 0:2 * W512],
                                      wo[h2 * P:(h2 + 1) * P, 0:2 * W512])
                    nc.sync.dma_start(t[:, 2 * W512:],
                                      wo[h2 * P:(h2 + 1) * P, 2 * W512:])
                    wot.append(t)

                # cover the wo DMA with pending attention work
                while not dm_done[2][0] and pending:
                    pump(1)
                for sc in range(NSC):
                    wi = sc // 4
                    # oTt[3][:, sc-slice] requires dm(h3, wi)
                    while not dm_done[wi][0]:
                        assert pending, f"stream exhausted before dm({wi})"
                        pump(1)
                    ost = ost_pool.tile([P, E], BF16, tag="ost", name=f"ost_{sc}")
                    for w in range(NW):
                        ws = slice(w * W512, (w + 1) * W512)
                        pso = ps_out.tile([P, W512], F32, tag="po", name=f"psout_{sc}_{w}")
                        for hh in range(HL):
                            nc.tensor.matmul(pso[:],
                                             oTt[hh][:, sc * P:(sc + 1) * P],
                                             wot[hh][:, ws],
                                             start=(hh == 0), stop=(hh == HL - 1))
                            if hh == 2:
                                pump(1)
                        nc.scalar.copy(ost[:, ws], pso[:])
                        if w == 1:
                            nc.sync.dma_start(
                                outp[sc * P:(sc + 1) * P, 0:2 * W512],
                                ost[:, 0:2 * W512])
                        elif sc == NSC - 1 and w == 2:
                            # final chunk: drip the stores out per-window so
                            # the tail after the last matmul is short
                            nc.sync.dma_start(
                                outp[sc * P:(sc + 1) * P, 2 * W512:3 * W512],
                                ost[:, 2 * W512:3 * W512])
                    if sc == NSC - 1:
                        nc.sync.dma_start(outp[sc * P:(sc + 1) * P, 3 * W512:],
                                          ost[:, 3 * W512:])
                    else:
                        nc.sync.dma_start(outp[sc * P:(sc + 1) * P, 2 * W512:],
                                          ost[:, 2 * W512:])
                drain()

    nc.compile()
    return nc


def _get_program():
    global _PROGRAM
    if _PROGRAM is None:
        _PROGRAM = _build_program()
    return _PROGRAM


def _wqk_layout(w):
    # [E, HL*D] -> [HL, P, NE*D]: w[e*P+p, h*D+c] -> out[h, p, e*D+c]
    return np.ascontiguousarray(
        w.reshape(NE, P, HL, D).transpose(2, 1, 0, 3).reshape(HL, P, NE * D))


def _host_prep(x, Wqkv, Wo, freqs_cis):
    """Build the 8 per-core input maps."""
    bf16 = ml_dtypes.bfloat16
    x = np.asarray(x, dtype=np.float32)
    Wqkv = np.asarray(Wqkv, dtype=np.float32)
    Wo = np.asarray(Wo, dtype=np.float32)
    freqs_cis = np.asarray(freqs_cis, dtype=np.float32)

    scale = np.float32(D ** -0.5)
    cos = freqs_cis[:, 0, :, 0].T        # [64, S]
    sin = freqs_cis[:, 0, :, 1].T
    cosf = np.ascontiguousarray(np.repeat(cos, 2, axis=0)).astype(bf16)
    # sign of the rotate-half term folded in: s[2i] = +sin_i, s[2i+1] = -sin_i,
    # so swap(raw*s) == swap(raw)*sin with the RoPE signs built in and the
    # swap itself becomes a pure partition-pair permutation (done via DMA)
    sinf = np.repeat(sin, 2, axis=0)
    sinf[1::2, :] *= -1.0
    sinf = np.ascontiguousarray(sinf).astype(bf16)

    # mask tiles: maskT[k, r*512 + q] = 0 if k + 128r <= q else -1e30
    kk = np.arange(P)[:, None]
    qq = np.arange(W512)[None, :]
    maskT = np.concatenate(
        [np.where(kk + P * r <= qq, 0.0, NEG).astype(np.float32)
         for r in range(4)],
        axis=1)
    maskT = np.ascontiguousarray(maskT).astype(bf16)

    xTb = [np.ascontiguousarray(x[b].T).astype(bf16) for b in range(B)]
    xq1b = [np.ascontiguousarray(t[:, 0:512]) for t in xTb]
    xq2b = [np.ascontiguousarray(t[:, 512:1024]) for t in xTb]
    xh2b = [np.ascontiguousarray(t[:, 1024:2048]) for t in xTb]

    in_maps = []
    for core in range(8):
        b, hg = divmod(core, 4)
        cs = slice(hg * 512, (hg + 1) * 512)
        in_maps.append({
            "xq1": xq1b[b],
            "xq2": xq2b[b],
            "xh2": xh2b[b],
            "wq": _wqk_layout(Wqkv[:, 0 * E:1 * E][:, cs] * scale).astype(bf16),
            "wk": _wqk_layout(Wqkv[:, 1 * E:2 * E][:, cs]).astype(bf16),
            "wv": np.ascontiguousarray(Wqkv[:, 2 * E:3 * E][:, cs]).astype(bf16),
            "wo": np.ascontiguousarray(Wo[hg * 512:(hg + 1) * 512, :]).astype(bf16),
            "cosf": cosf,
            "sinf": sinf,
            "maskT": maskT,
        })
    return in_maps


def run_cores(x, Wqkv, Wo, freqs_cis, trace=False, **kw):
    """Run the 8-core SPMD program; returns (partials list, BassKernelResults)."""
    nc = _get_program()
    in_maps = _host_prep(x, Wqkv, Wo, freqs_cis)
    res = bass_utils.run_bass_kernel_spmd(
        nc, in_maps, core_ids=list(range(8)), trace=trace, **kw)
    return [np.asarray(r["outp"], dtype=np.float32) for r in res.results], res


def kernel(x, Wqkv, Wo, freqs_cis):
    partials, _ = run_cores(x, Wqkv, Wo, freqs_cis)
    out = np.empty((B, S, E), dtype=np.float32)
    for b in range(B):
        acc = partials[4 * b]
        for hg in range(1, 4):
            acc = acc + partials[4 * b + hg]
        out[b] = acc
    return out



# revision 27
# speedup vs baseline: 1.0074x; 1.0074x over previous
"""Causal self-attention (B=2, S=2048, E=2048, H=16) on 8 TRN2 NeuronCores.

Sharding: core = 4*b + hg  (b in {0,1} data-parallel over batch,
hg in {0..3} tensor-parallel over groups of 4 heads / Wqkv columns /
Wo rows).  Each core computes a partial [S, E] output (its heads'
contribution through Wo); the host sums the 4 partials per batch.

v2 design (all-bf16, SBUF-resident, software-pipelined):
  - Every matmul operand is bf16 (PSUM accumulation stays fp32), which
    halves DMA traffic + SBUF footprint vs f32r and enables FWL (2x
    faster LDWEIGHTS), keeping the weight path off the PE critical path.
  - No DRAM spill: x, q/k (post-RoPE), v, oT all live in SBUF; q/k
    weights are host-pre-transposed so every DMA is contiguous.
  - Softmax denominator is NOT computed with per-block ones-matmuls
    (which would double the AV cost on PE).  Exp blocks are summed by a
    sequential bf16 chain on the DVE (each 438ns link is faster than
    the ~660ns exp cadence, so the chain never lags), then one small
    bf16 ones-matmul per (head, window) broadcasts the partition sum.
  - RoPE: rot = raw*cos + pairswap(raw*sin_signed), with the rotate-
    half signs folded into sin on the host so the swap is a pure
    partition-pair permutation done by two strided SBUF->SBUF DMAs —
    zero PE cost.
  - Software pipelining: attention of head h (ACT-heavy exp, PE-light)
    is emitted as a thunk stream interleaved into the projection matmul
    stream of head h+1 and the final Wo stream, so the PE never drains
    while the scalar engine chews through exp.  Measured: single >0.5us
    PE gap over the whole kernel outside the DMA preamble/tail.
"""
import sys

if "/opt/trn_rl_repo" not in sys.path:
    sys.path.insert(0, "/opt/trn_rl_repo")

from collections import deque
from contextlib import ExitStack

import numpy as np
import ml_dtypes

import concourse.bass as bass
import concourse.mybir as mybir
import concourse.tile as tile
from concourse import bacc, bass_utils

F32 = mybir.dt.float32
F32R = mybir.dt.float32r
BF16 = mybir.dt.bfloat16
AF = mybir.ActivationFunctionType
ALU = mybir.AluOpType

B = 2
S = 2048
E = 2048
H = 16
D = 128
HL = 4            # heads per core
P = 128
NE = E // P       # 16 contraction chunks
NW = S // 512     # 4 q windows of 512
NSC = S // P      # 16 s-chunks
W512 = 512
NEG = -1.0e30

_PROGRAM = None


def _build_program():
    nc = bacc.Bacc("TRN2", target_bir_lowering=False, debug=False, num_devices=8)

    # x^T split column-wise into contiguous host tensors so the v-phase
    # e-loop is PE-bound, not DMA-bound (first pass needs only [0:512])
    xq1 = nc.dram_tensor("xq1", [E, W512], BF16, kind="ExternalInput").ap()
    xq2 = nc.dram_tensor("xq2", [E, W512], BF16, kind="ExternalInput").ap()
    xh2 = nc.dram_tensor("xh2", [E, 2 * W512], BF16, kind="ExternalInput").ap()
    # pre-transposed on host: wq[h, p, e*D+c] = Wq_scaled[e*P+p, h*D+c]
    wq = nc.dram_tensor("wq", [HL, P, NE * D], BF16, kind="ExternalInput").ap()
    wk = nc.dram_tensor("wk", [HL, P, NE * D], BF16, kind="ExternalInput").ap()
    wv = nc.dram_tensor("wv", [E, HL * D], BF16, kind="ExternalInput").ap()
    wo = nc.dram_tensor("wo", [HL * D, E], BF16, kind="ExternalInput").ap()
    cosf = nc.dram_tensor("cosf", [P, S], BF16, kind="ExternalInput").ap()
    sinf = nc.dram_tensor("sinf", [P, S], BF16, kind="ExternalInput").ap()
    # single [P, P] causal triangle: the diagonal block k+128r <= q at
    # q = 128r + j reduces to k <= j, identical for every r
    maskT = nc.dram_tensor("maskT", [P, P], BF16, kind="ExternalInput").ap()
    outp = nc.dram_tensor("outp", [S, E], BF16, kind="ExternalOutput").ap()

    with tile.TileContext(nc) as tc:
        with ExitStack() as ctx_all:
            cpool = ctx_all.enter_context(tc.tile_pool(name="const", bufs=1))
            ones_t = cpool.tile([P, P], BF16, tag="ones")
            # ones synthesized on-chip (no DMA)
            nc.gpsimd.memset(ones_t[:], 1.0)

            vsb_pool = ctx_all.enter_context(tc.tile_pool(name="vsb", bufs=1))
            vsb = vsb_pool.tile([P, NSC, HL, D], BF16, tag="v")
            qk_pool = ctx_all.enter_context(tc.tile_pool(name="qk", bufs=1))
            qT = [qk_pool.tile([P, S], BF16, tag=f"q{h}", name=f"qT_{h}") for h in range(HL)]
            kT = [qk_pool.tile([P, S], BF16, tag=f"k{h}", name=f"kT_{h}") for h in range(HL)]
            oT_pool = ctx_all.enter_context(tc.tile_pool(name="oT", bufs=1))
            oTt = [oT_pool.tile([P, S], BF16, tag=f"o{h}", name=f"oT_{h}") for h in range(HL)]

            # ------- attention-stream machinery (emitted via pump()) -------
            pending = deque()

            def pump(n=1):
                for _ in range(n):
                    if not pending:
                        return
                    pending.popleft()()

            def drain():
                while pending:
                    pending.popleft()()

            # the following pools are entered later (after the v-phase frees
            # its SBUF); placeholders filled by alloc_attn_pools()
            attn = {}

            def alloc_attn_pools():
                attn["mask_pool"] = ctx_all.enter_context(
                    tc.tile_pool(name="mask", bufs=1, side="right"))
                attn["mask_t"] = attn["mask_pool"].tile([P, P], BF16,
                                                        tag="mask", name="mask_t")
                nc.sync.dma_start(attn["mask_t"][:], maskT)
                attn["expw_pool"] = ctx_all.enter_context(
                    tc.tile_pool(name="expw", bufs=1, side="right"))
                attn["acc_pool"] = ctx_all.enter_context(
                    tc.tile_pool(name="acc", bufs=1, side="right"))
                attn["rec_pool"] = ctx_all.enter_context(
                    tc.tile_pool(name="rec", bufs=1, side="right"))
                attn["ps_s"] = ctx_all.enter_context(
                    tc.tile_pool(name="ps_s", bufs=4, space="PSUM", side="right"))
                attn["ps_o"] = ctx_all.enter_context(
                    tc.tile_pool(name="ps_o", bufs=2, space="PSUM", side="right"))

            # per-head transient state for the in-flight stream
            st = {}

            def op_score(h, w, c):
                def f():
                    r = c - 4 * w
                    ex0 = max(0, P * r)
                    pss = attn["ps_s"].tile([P, W512], F32, tag="s", name=f"pss_{h}_{w}_{c}")
                    nc.tensor.matmul(
                        pss[:, ex0:], kT[h][:, c * P:(c + 1) * P],
                        qT[h][:, w * W512 + ex0:(w + 1) * W512],
                        start=True, stop=True)
                    if r >= 0:
                        nc.vector.tensor_add(
                            pss[:, ex0:ex0 + P], pss[:, ex0:ex0 + P],
                            attn["mask_t"][:])
                    expw = st[(h, "expw", w)]
                    if ex0 > 0:
                        nc.gpsimd.memset(expw[:, c, 0:ex0], 0.0)
                    nc.scalar.activation(expw[:, c, ex0:], pss[:, ex0:], AF.Exp)
                    # denominator partial sum: sequential bf16 chain on DVE
                    # (438ns/link beats the ~660ns exp cadence, so the chain
                    # never lags the exp stream)
                    if c == 0:
                        acc = attn["acc_pool"].tile([P, W512], BF16, tag="a",
                                                    name=f"acc_{h}_{w}")
                        st[(h, "acc", w)] = acc
                        nc.vector.tensor_copy(acc[:], expw[:, 0, :])
                    else:
                        acc = st[(h, "acc", w)]
                        nc.vector.tensor_add(acc[:], acc[:], expw[:, c, :])
                return f

            def op_expw_alloc(h, w):
                def f():
                    # parity-shared tags (cap 12 even / 16 odd) so the pool
                    # holds 28KB/part instead of 40; only 4w+4 chunks used
                    cap = 16 if w % 2 else 12
                    st[(h, "expw", w)] = attn["expw_pool"].tile(
                        [P, cap, W512], BF16, tag=f"e{w % 2}",
                        name=f"expw_{h}_{w}")
                return f

            def op_av(h, w, c):
                def f():
                    nblk = 4 * w + 4
                    r = c - 4 * w
                    ex0 = max(0, P * r)
                    if c == 0:
                        st[(h, "pso", w)] = attn["ps_o"].tile(
                            [P, W512], F32, tag="o", name=f"pso_{h}_{w}")
                    pso = st[(h, "pso", w)]
                    expw = st[(h, "expw", w)]
                    nc.tensor.matmul(
                        pso[:, ex0:], vsb[:, c, h, :], expw[:, c, ex0:],
                        start=(c == 0), stop=(c == nblk - 1))
                return f

            def op_dm(h, w, done_marker=None):
                def f():
                    acc = st.pop((h, "acc", w))
                    pso = st.pop((h, "pso", w))
                    psd = attn["ps_s"].tile([P, W512], F32, tag="s", name=f"psd_{h}_{w}")
                    nc.tensor.matmul(psd[:], ones_t[:], acc[:],
                                     start=True, stop=True)
                    recd = attn["rec_pool"].tile([P, W512], F32, tag="r", name=f"recd_{h}_{w}")
                    nc.vector.reciprocal_approx_fast(out=recd[:], in_=psd[:])
                    nc.vector.tensor_mul(
                        oTt[h][:, w * W512:(w + 1) * W512], pso[:], recd[:])
                    if done_marker is not None:
                        done_marker[0] = True
                return f

            # dm_done[w]: Wo phase drains the h3 stream until these flip
            dm_done = {w: [False] for w in range(NW)}

            def stream_ops_for_window(h, w, mark):
                """ops that become emittable once proj window w of head h
                is complete (rot written).  Returns list of thunks."""
                ops = []
                ops.append(op_expw_alloc(h, w))
                for c in range(4 * w + 4):
                    ops.append(op_score(h, w, c))
                if w >= 1:
                    wp = w - 1
                    for c in range(4 * wp + 4):
                        ops.append(op_av(h, wp, c))
                    ops.append(op_dm(h, wp, mark[wp] if mark else None))
                if w == NW - 1:
                    for c in range(4 * w + 4):
                        ops.append(op_av(h, w, c))
                    ops.append(op_dm(h, w, mark[w] if mark else None))
                return ops

            def enqueue_head(h, mark=None):
                for w in range(NW):
                    pending.extend(stream_ops_for_window(h, w, mark))

            # ---------------- Phase 1: QKV projection + RoPE ----------------
            with ExitStack() as ctx1:
                xpool = ctx1.enter_context(tc.tile_pool(name="xT", bufs=NE))
                cspool = ctx1.enter_context(tc.tile_pool(name="cossin", bufs=1))
                wqk_pool = ctx1.enter_context(tc.tile_pool(name="wqk", bufs=4))
                rsin_pool = ctx1.enter_context(tc.tile_pool(name="rsin", bufs=2))

                xt = []
                with ExitStack() as ctxv:
                    wv_pool = ctxv.enter_context(tc.tile_pool(name="wv", bufs=NE))
                    ps_v = ctxv.enter_context(
                        tc.tile_pool(name="ps_v", bufs=8, space="PSUM"))
                    # triple-interleave (wv_e, xq1_e, xq2_e) so the 8-chunk
                    # v pass (1.73us of PE per triple) chases the DMA stream
                    # PE-bound instead of DMA-starved
                    wvt = []
                    for e in range(NE):
                        tw = wv_pool.tile([P, HL * D], BF16, tag="wv",
                                          name=f"wv_{e}")
                        nc.sync.dma_start(tw[:], wv[e * P:(e + 1) * P, :])
                        wvt.append(tw)
                        t1 = xpool.tile([P, W512], BF16, tag="x1", name=f"x1_{e}")
                        nc.sync.dma_start(t1[:], xq1[e * P:(e + 1) * P, :])
                        t2 = xpool.tile([P, W512], BF16, tag="x2", name=f"x2_{e}")
                        nc.sync.dma_start(t2[:], xq2[e * P:(e + 1) * P, :])
                        xt.append([t1, t2, None])
                    for e in range(NE):
                        t3 = xpool.tile([P, 2 * W512], BF16, tag="x3",
                                        name=f"x3_{e}")
                        nc.sync.dma_start(t3[:], xh2[e * P:(e + 1) * P, :])
                        xt[e][2] = t3
                    # q/k weights for head 0 prefetched during the v-phase
                    wt_qk = {}
                    for m, wsrc in ((0, wq), (1, wk)):
                        t = wqk_pool.tile([P, NE, P], BF16, tag="wqk", name=f"wt_0_{m}")
                        nc.sync.dma_start(
                            t[:].rearrange("p n c -> p (n c)"), wsrc[0])
                        wt_qk[(0, m)] = t
                    cos_t = cspool.tile([P, S], BF16, tag="cos")
                    nc.sync.dma_start(cos_t[:], cosf)
                    sin_t = cspool.tile([P, S], BF16, tag="sin")
                    nc.sync.dma_start(sin_t[:], sinf)

                    # PE warm-up: dummy matmuls with no DMA dependency ramp
                    # the clock through the DMA-init dead zone so the first
                    # real matmul runs at full p-state.
                    warm_t = wv_pool.tile([P, W512], BF16, tag="warm",
                                          name="warm_t")
                    nc.gpsimd.memset(warm_t[:], 1.0)
                    ps_warm = ps_v.tile([P, W512], F32, tag="v", name="ps_warm")
                    for _ in range(7):
                        nc.tensor.matmul(ps_warm[:], warm_t[:, 0:P], warm_t[:],
                                         start=True, stop=True)

                    # v: passes of 8/6/2 s-chunks (pool-close barrier waits
                    # for ALL psum->vsb copies, so the final pass is tiny and
                    # its 2 copies are split into half-copies on both engines
                    # to minimize the trailing work after the last v matmul).
                    # First/last accumulation rounds run ci-descending so the
                    # banks the next pass grabs free earliest.
                    for g, (c0, nci) in enumerate(((0, 8), (8, 6), (14, 2))):
                        psvs = [ps_v.tile([P, W512], F32, tag="v",
                                          name=f"psv_{g}_{ci}")
                                for ci in range(nci)]
                        for e in range(NE):
                            cis = (range(nci - 1, -1, -1)
                                   if e in (0, NE - 1) else range(nci))
                            for ci in cis:
                                c = c0 + ci
                                if c < 4:
                                    xs = xt[e][0][:, c * P:(c + 1) * P]
                                elif c < 8:
                                    xs = xt[e][1][:, (c - 4) * P:(c - 3) * P]
                                else:
                                    xs = xt[e][2][:, (c - 8) * P:(c - 7) * P]
                                nc.tensor.matmul(
                                    psvs[ci][:], xs, wvt[e][:],
                                    start=(e == 0), stop=(e == NE - 1))
                        for ci in range(nci - 1, -1, -1):
                            c = c0 + ci
                            # [s, 4h*128] -> vsb[:, c, h, :]
                            if g == 2:
                                # half-copies on both engines: shortest tail
                                nc.scalar.copy(
                                    vsb[:, c, 0:2, :].rearrange("p h d -> p (h d)"),
                                    psvs[ci][:, 0:2 * D])
                                nc.vector.tensor_copy(
                                    vsb[:, c, 2:4, :].rearrange("p h d -> p (h d)"),
                                    psvs[ci][:, 2 * D:])
                            else:
                                dstv = vsb[:, c, :, :].rearrange("p h d -> p (h d)")
                                eng = (nc.scalar.copy,
                                       nc.vector.tensor_copy)[(nci - 1 - ci) % 2]
                                eng(dstv, psvs[ci][:])

                # v-phase SBUF/PSUM released.  attn pools are allocated one
                # window into head 0's projection (their gpsimd zero-fills and
                # the mask DMA then overlap qk matmuls instead of gating the
                # first one); only ps_qk/swp are needed immediately.
                ps_qk = ctx1.enter_context(
                    tc.tile_pool(name="ps_qk", bufs=2, space="PSUM"))
                swp_pool = ctx1.enter_context(tc.tile_pool(name="swp", bufs=2))

                # q, k projection with RoPE; per head: w-outer, m-inner.
                # rope "finish" (swap-matmul + combine) for window w is
                # deferred into window w+1's e-loop so the PE never waits
                # on the DVE.
                for h in range(HL):
                    rope_dma_q = deque()   # (rs, dst) awaiting the swap DMA
                    rope_add_q = deque()   # (swp, dst) awaiting the combine

                    def rope_dma():
                        rs_keep, dst = rope_dma_q.popleft()
                        swp = swp_pool.tile([P, W512], BF16, tag="sw",
                                            name="swp")
                        nc.sync.dma_start(swp[0::2, :], rs_keep[1::2, :])
                        nc.sync.dma_start(swp[1::2, :], rs_keep[0::2, :])
                        rope_add_q.append((swp, dst))

                    def rope_add():
                        swp, dst = rope_add_q.popleft()
                        nc.vector.tensor_add(dst, dst, swp[:])

                    def xwin(e, w):
                        if w == 0:
                            return xt[e][0][:]
                        if w == 1:
                            return xt[e][1][:]
                        return xt[e][2][:, (w - 2) * W512:(w - 1) * W512]

                    for w in range(NW):
                        ws = slice(w * W512, (w + 1) * W512)
                        for m in range(2):
                            wt = wt_qk[(h, m)]
                            ps = ps_qk.tile([P, W512], F32, tag="qk", name=f"psqk_{h}_{w}_{m}")
                            for e in range(NE):
                                nc.tensor.matmul(ps[:], wt[:, e, :],
                                                 xwin(e, w),
                                                 start=(e == 0),
                                                 stop=(e == NE - 1))
                                if e in (1, 2) and rope_dma_q:
                                    rope_dma()
                                elif e in (8, 9) and rope_add_q:
                                    rope_add()
                                elif e > 0:
                                    pump(2 if len(pending) > 50 else 1)
                            rs = rsin_pool.tile([P, W512], BF16, tag="rs", name=f"rs_{h}_{w}_{m}")
                            nc.vector.tensor_mul(rs[:], ps[:], sin_t[:, ws])
                            dst = (qT[h] if m == 0 else kT[h])[:, ws]
                            nc.vector.tensor_mul(dst, ps[:], cos_t[:, ws])
                            rope_dma_q.append((rs, dst))
                        if h == 0 and w == 0:
                            alloc_attn_pools()
                        if h == HL - 1 and w == 0:
                            # prefetch Wo during h3's projection so the Wo
                            # phase's first matmuls aren't DMA-gated
                            attn["wo_pool"] = ctx_all.enter_context(
                                tc.tile_pool(name="wo", bufs=1, side="right"))
                            wot_pref = []
                            for h2 in range(HL):
                                t = attn["wo_pool"].tile(
                                    [P, E], BF16, tag=f"wo{h2}", name=f"wo_{h2}")
                                nc.sync.dma_start(
                                    t[:, 0:2 * W512],
                                    wo[h2 * P:(h2 + 1) * P, 0:2 * W512])
                                nc.sync.dma_start(
                                    t[:, 2 * W512:],
                                    wo[h2 * P:(h2 + 1) * P, 2 * W512:])
                                wot_pref.append(t)
                            attn["wot"] = wot_pref
                        # prefetch next head's weights well before its start
                        if h + 1 < HL and w == NW - 2:
                            for m2, wsrc in ((0, wq), (1, wk)):
                                t = wqk_pool.tile([P, NE, P], BF16, tag="wqk", name=f"wt_{h+1}_{m2}")
                                nc.sync.dma_start(
                                    t[:].rearrange("p n c -> p (n c)"),
                                    wsrc[h + 1])
                                wt_qk[(h + 1, m2)] = t
                        # head 3's attention ops become valid window by
                        # window (rope of window w finishes during window
                        # w+1): enqueue them as soon as they are legal so
                        # they drain during head 3's own projection.
                        if h == HL - 1 and w >= 1:
                            pending.extend(
                                stream_ops_for_window(h, w - 1, dm_done))
                    # enqueue head h's attention BEFORE the rope flush so the
                    # flush's pumps feed the PE real work (safe: the flush
                    # only pumps the front of pending = this head's w0 ops,
                    # whose rope finished during w1).  h3's last window must
                    # stay after the flush: its scores read the w3 rope that
                    # the flush itself emits.
                    if h < HL - 1:
                        enqueue_head(h)
                    # flush the last window's rope
                    while rope_dma_q:
                        rope_dma()
                        pump(1)
                    while rope_add_q:
                        pump(2)
                        rope_add()
                    if h == HL - 1:
                        pending.extend(
                            stream_ops_for_window(h, NW - 1, dm_done))

            # ---------------- Phase 3: Wo ----------------
            with ExitStack() as ctx3:
                ost_pool = ctx3.enter_context(tc.tile_pool(name="ost", bufs=2))
                ps_out = ctx3.enter_context(
                    tc.tile_pool(name="ps_out", bufs=2, space="PSUM"))
                wot = attn["wot"]

                # cover the wo DMA with pending attention work
                while not dm_done[2][0] and pending:
                    pump(1)
                for sc in range(NSC):
                    wi = sc // 4
                    # oTt[3][:, sc-slice] requires dm(h3, wi)
                    while not dm_done[wi][0]:
                        assert pending, f"stream exhausted before dm({wi})"
                        pump(1)
                    ost = ost_pool.tile([P, E], BF16, tag="ost", name=f"ost_{sc}")
                    for w in range(NW):
                        ws = slice(w * W512, (w + 1) * W512)
                        pso = ps_out.tile([P, W512], F32, tag="po", name=f"psout_{sc}_{w}")
                        for hh in range(HL):
                            nc.tensor.matmul(pso[:],
                                             oTt[hh][:, sc * P:(sc + 1) * P],
                                             wot[hh][:, ws],
                                             start=(hh == 0), stop=(hh == HL - 1))
                            if hh == 2:
                                pump(1)
                        nc.scalar.copy(ost[:, ws], pso[:])
                        if w == 1:
                            nc.sync.dma_start(
                                outp[sc * P:(sc + 1) * P, 0:2 * W512],
                                ost[:, 0:2 * W512])
                        elif sc == NSC - 1 and w == 2:
                            # final chunk: drip the stores out per-window so
                            # the tail after the last matmul is short
                            nc.sync.dma_start(
                                outp[sc * P:(sc + 1) * P, 2 * W512:3 * W512],
                                ost[:, 2 * W512:3 * W512])
                    if sc == NSC - 1:
                        nc.sync.dma_start(outp[sc * P:(sc + 1) * P, 3 * W512:],
                                          ost[:, 3 * W512:])
                    else:
                        nc.sync.dma_start(outp[sc * P:(sc + 1) * P, 2 * W512:],
                                          ost[:, 2 * W512:])
                drain()

    nc.compile()
    return nc


def _get_program():
    global _PROGRAM
    if _PROGRAM is None:
        _PROGRAM = _build_program()
    return _PROGRAM


def _wqk_layout(w):
    # [E, HL*D] -> [HL, P, NE*D]: w[e*P+p, h*D+c] -> out[h, p, e*D+c]
    return np.ascontiguousarray(
        w.reshape(NE, P, HL, D).transpose(2, 1, 0, 3).reshape(HL, P, NE * D))


def _host_prep(x, Wqkv, Wo, freqs_cis):
    """Build the 8 per-core input maps."""
    bf16 = ml_dtypes.bfloat16
    x = np.asarray(x, dtype=np.float32)
    Wqkv = np.asarray(Wqkv, dtype=np.float32)
    Wo = np.asarray(Wo, dtype=np.float32)
    freqs_cis = np.asarray(freqs_cis, dtype=np.float32)

    scale = np.float32(D ** -0.5)
    cos = freqs_cis[:, 0, :, 0].T        # [64, S]
    sin = freqs_cis[:, 0, :, 1].T
    cosf = np.ascontiguousarray(np.repeat(cos, 2, axis=0)).astype(bf16)
    # sign of the rotate-half term folded in: s[2i] = +sin_i, s[2i+1] = -sin_i,
    # so swap(raw*s) == swap(raw)*sin with the RoPE signs built in and the
    # swap itself becomes a pure partition-pair permutation (done via DMA)
    sinf = np.repeat(sin, 2, axis=0)
    sinf[1::2, :] *= -1.0
    sinf = np.ascontiguousarray(sinf).astype(bf16)

    # single diagonal-block mask: maskT[k, j] = 0 if k <= j else -1e30
    kk = np.arange(P)[:, None]
    jj = np.arange(P)[None, :]
    maskT = np.ascontiguousarray(
        np.where(kk <= jj, 0.0, NEG).astype(np.float32)).astype(bf16)

    xTb = [np.ascontiguousarray(x[b].T).astype(bf16) for b in range(B)]
    xq1b = [np.ascontiguousarray(t[:, 0:512]) for t in xTb]
    xq2b = [np.ascontiguousarray(t[:, 512:1024]) for t in xTb]
    xh2b = [np.ascontiguousarray(t[:, 1024:2048]) for t in xTb]

    in_maps = []
    for core in range(8):
        b, hg = divmod(core, 4)
        cs = slice(hg * 512, (hg + 1) * 512)
        in_maps.append({
            "xq1": xq1b[b],
            "xq2": xq2b[b],
            "xh2": xh2b[b],
            "wq": _wqk_layout(Wqkv[:, 0 * E:1 * E][:, cs] * scale).astype(bf16),
            "wk": _wqk_layout(Wqkv[:, 1 * E:2 * E][:, cs]).astype(bf16),
            "wv": np.ascontiguousarray(Wqkv[:, 2 * E:3 * E][:, cs]).astype(bf16),
            "wo": np.ascontiguousarray(Wo[hg * 512:(hg + 1) * 512, :]).astype(bf16),
            "cosf": cosf,
            "sinf": sinf,
            "maskT": maskT,
        })
    return in_maps


def run_cores(x, Wqkv, Wo, freqs_cis, trace=False, **kw):
    """Run the 8-core SPMD program; returns (partials list, BassKernelResults)."""
    nc = _get_program()
    in_maps = _host_prep(x, Wqkv, Wo, freqs_cis)
    res = bass_utils.run_bass_kernel_spmd(
        nc, in_maps, core_ids=list(range(8)), trace=trace, **kw)
    return [np.asarray(r["outp"], dtype=np.float32) for r in res.results], res


def kernel(x, Wqkv, Wo, freqs_cis):
    partials, _ = run_cores(x, Wqkv, Wo, freqs_cis)
    out = np.empty((B, S, E), dtype=np.float32)
    for b in range(B):
        acc = partials[4 * b]
        for hg in range(1, 4):
            acc = acc + partials[4 * b + hg]
        out[b] = acc
    return out

